# revision 11
# baseline (speedup 1.0000x reference)
"""GAT (2-layer, 8 heads) + MLP on 8 Trainium2 NeuronCores — full Bass kernel.

Node-parallel layout (per sharding hint): nodes are relabeled host-side so
every 128-node "window" has balanced in-degree, then row-sharded 8 ways
(core c owns windows [c*WPC, (c+1)*WPC)). Per layer:
  dense:  Y = X @ Waug on the tensor engine (Waug folds the attention
          projections: each Y row is [h | a_src.h | a_dst.h]; the 16
          attention logits are stored f32-bitcast inside the bf16 row),
          replicated per core so the edge gather reads core-local HBM.
  edges:  MoE-style dma_gather pulls Y[src] rows for each core's incoming
          edges (dst-sorted, window-padded streams; src<32768 and >=32768
          gathered separately to fit int16 indices; labels 0 and ASPLIT
          are reserved zero-feature pad nodes so pad slots gather zeros).
          Softmax runs without max-subtraction (exp of small logits), so
          attention+aggregation collapse to one weighted scatter:
          out[n] = sum_e w_e*[h_src|1], computed as one-hot matmuls into
          PSUM per window. Each window's stream carries a leading "self
          subtile" (the window's own 128 rows, in order) in whichever
          stream its labels fall in — this supplies both the self-loop
          edges and the window's a_dst table.
  The inter-layer exchange (relu(out1) must be visible to all cores for
  layer-2's dense) is an in-kernel AllGather collective.
"""
import os
import sys
import time

for _p in ("/opt/trn_rl_repo",):
    if _p not in sys.path:
        sys.path.append(_p)

import numpy as np
import ml_dtypes

# model dims (fixed by the problem)
IN_CH, HID, GOUT, HEADS = 128, 32, 64, 8
MLP_HID, OUT_CLASSES, NEG = 64, 2, 0.2
C1, C2 = HEADS * HID, HEADS * GOUT          # 256, 512
RW1, RW2 = 384, 640                          # padded Y row widths (bf16)

BF16 = ml_dtypes.bfloat16


class Cfg:
    def __init__(self, N, NPAD, NC, WPC, ASPLIT, CAPA, CAPB, GRP, CHUNKS):
        self.N, self.NPAD, self.NC, self.WPC = N, NPAD, NC, WPC
        self.ASPLIT, self.CAPA, self.CAPB = ASPLIT, CAPA, CAPB
        self.NSA, self.NSB = CAPA // 128 + 1, CAPB // 128 + 1  # incl self subtile
        self.SLA, self.SLB = CAPA + 128, CAPB + 128            # slots per window
        self.GRP, self.CHUNKS = GRP, tuple(CHUNKS)
        self.NWIN = NPAD // 128
        assert NPAD == NC * WPC * 128 and WPC % GRP == 0 and sum(CHUNKS) == WPC
        assert ASPLIT % 128 == 0


FULL = Cfg(N=50000, NPAD=50176, NC=8, WPC=49, ASPLIT=32768,
           CAPA=768, CAPB=512, GRP=1, CHUNKS=(13, 12, 12, 12))


# ----------------------------------------------------------------------------
# host-side graph prep
# ----------------------------------------------------------------------------

def relabel(dst_noloop, cfg):
    deg = np.bincount(dst_noloop, minlength=cfg.N)
    deg_all = np.concatenate([deg, np.zeros(cfg.NPAD - cfg.N, np.int64)])
    order = np.argsort(-deg_all, kind="stable")
    win_of = np.empty(cfg.NPAD, np.int64)
    fwd = np.arange(cfg.NWIN)
    rev = fwd[::-1]
    nb = cfg.NPAD // cfg.NWIN
    for b in range(nb):
        idxs = order[b * cfg.NWIN : (b + 1) * cfg.NWIN]
        win_of[idxs] = fwd if b % 2 == 0 else rev
    lab = np.empty(cfg.NPAD, np.int64)
    perm = np.lexsort((np.arange(cfg.NPAD), win_of))
    lab[perm] = np.arange(cfg.NPAD)
    # reserve labels 0 and ASPLIT for zero-feature pad nodes (so index-0
    # pad slots in each stream gather zero rows)
    inv = np.argsort(lab)
    for want, padn in ((0, cfg.N), (cfg.ASPLIT, cfg.N + 1)):
        holder = inv[want]
        if holder >= cfg.N:
            continue
        lab[holder], lab[padn] = lab[padn], lab[holder]
        inv = np.argsort(lab)
    return lab


def build_streams(src, dst, lab, cfg):
    """src/dst: random edges only. Self-loops become per-window self
    subtiles (slots [0:128] of the window's A or B stream)."""
    sl = lab[src]
    dl = lab[dst]
    order = np.argsort(dl, kind="stable")
    sl, dl = sl[order], dl[order]
    w = (dl >> 7).astype(np.int64)
    ld = (dl & 127).astype(np.int64)
    isA = sl < cfg.ASPLIT
    wstarts = np.searchsorted(w, np.arange(cfg.NWIN + 1))
    cores = []
    ar = np.arange(128)
    for c in range(cfg.NC):
        idxA = np.zeros((cfg.WPC, cfg.SLA), np.int16)
        idxB = np.zeros((cfg.WPC, cfg.SLB), np.int16)
        ldA = np.full((cfg.WPC, cfg.SLA), -1.0, np.float32)
        ldB = np.full((cfg.WPC, cfg.SLB), -1.0, np.float32)
        csrA = np.zeros((cfg.WPC, 128, 2), np.float32)
        csrB = np.zeros((cfg.WPC, 128, 2), np.float32)
        for wi in range(cfg.WPC):
            gw = c * cfg.WPC + wi
            base = gw * 128
            if base < cfg.ASPLIT:
                idxA[wi, 0:128] = (base + ar).astype(np.int16)
                ldA[wi, 0:128] = ar
            else:
                idxB[wi, 0:128] = (base - cfg.ASPLIT + ar).astype(np.int16)
                ldB[wi, 0:128] = ar
            s0, s1 = wstarts[gw], wstarts[gw + 1]
            wsl, wld, wA = sl[s0:s1], ld[s0:s1], isA[s0:s1]
            for stream, cap, idx_o, ld_o, csr_o in (
                (True, cfg.CAPA, idxA, ldA, csrA),
                (False, cfg.CAPB, idxB, ldB, csrB),
            ):
                mm = wA == stream
                s_ids = wsl[mm] if stream else wsl[mm] - cfg.ASPLIT
                l_ids = wld[mm]
                cnt = len(s_ids)
                assert cnt <= cap, f"stream overflow: core {c} win {wi} {cnt}>{cap}"
                idx_o[wi, 128 : 128 + cnt] = s_ids.astype(np.int16)
                ld_o[wi, 128 : 128 + cnt] = l_ids
                csr_o[wi, :, 0] = np.searchsorted(l_ids, ar, side="left")
                csr_o[wi, :, 1] = np.searchsorted(l_ids, ar, side="right")
        cores.append(dict(idxA=idxA, idxB=idxB, ldA=ldA, ldB=ldB,
                          csrA=csrA, csrB=csrB))
    return cores


def pack_core(st, cfg):
    def wrap_idx(a, slots):
        w16 = (a.reshape(cfg.WPC, slots // 16, 16).transpose(2, 0, 1)
               .reshape(16, cfg.WPC * (slots // 16)))
        return np.tile(w16, (8, 1)).copy()

    def wrap_ld(a, ns):
        return (a.reshape(cfg.WPC, ns, 128).transpose(2, 0, 1)
                .reshape(128, cfg.WPC * ns)).astype(BF16).copy()

    def wrap_csr(a):
        return (a.transpose(1, 0, 2).reshape(128, cfg.WPC * 2)
                .astype(np.float16).copy())

    return dict(
        idxA=wrap_idx(st["idxA"], cfg.SLA),
        idxB=wrap_idx(st["idxB"], cfg.SLB),
        ldA=wrap_ld(st["ldA"], cfg.NSA),
        ldB=wrap_ld(st["ldB"], cfg.NSB),
        csrA=wrap_csr(st["csrA"]),
        csrB=wrap_csr(st["csrB"]),
    )


def _blockdiag(a, ch):
    B = np.zeros((HEADS * ch, HEADS), np.float32)
    for hd in range(HEADS):
        B[hd * ch : (hd + 1) * ch, hd] = a[hd]
    return B


# ----------------------------------------------------------------------------
# device program
# ----------------------------------------------------------------------------

def build_nc(cfg, part="all"):
    from concourse import bass, mybir
    import concourse.bacc as bacc
    import concourse.tile as tile
    import concourse.masks as masks

    bf = mybir.dt.bfloat16
    f32 = mybir.dt.float32
    f16 = mybir.dt.float16
    i16 = mybir.dt.int16
    AF = mybir.ActivationFunctionType
    OP = mybir.AluOpType

    nc = bacc.Bacc(num_devices=cfg.NC)
    SH = cfg.WPC * 128

    xT = nc.dram_tensor("xT", [IN_CH, cfg.NPAD], bf, kind="ExternalInput")
    w1 = nc.dram_tensor("w1", [IN_CH, C1 + 16], bf, kind="ExternalInput")
    b1r = nc.dram_tensor("b1r", [128, C1], f32, kind="ExternalInput")
    Y1 = nc.dram_tensor("Y1", [cfg.NPAD, RW1], mybir.dt.uint16)
    w2 = nc.dram_tensor("w2", [C1, C2 + 16], bf, kind="ExternalInput")
    b2r = nc.dram_tensor("b2r", [128, GOUT], f32, kind="ExternalInput")
    wm1 = nc.dram_tensor("wm1", [GOUT, MLP_HID], bf, kind="ExternalInput")
    wm2 = nc.dram_tensor("wm2", [MLP_HID, OUT_CLASSES], bf, kind="ExternalInput")
    bm1c = nc.dram_tensor("bm1c", [MLP_HID, 1], f32, kind="ExternalInput")
    bm2c = nc.dram_tensor("bm2c", [OUT_CLASSES, 1], f32, kind="ExternalInput")
    Y2 = nc.dram_tensor("Y2", [cfg.NPAD, RW2], mybir.dt.uint16)
    outT = nc.dram_tensor("outT", [OUT_CLASSES, SH], f32, kind="ExternalOutput")

    idxA = nc.dram_tensor("idxA", [128, cfg.WPC * cfg.SLA // 16], i16, kind="ExternalInput")
    idxB = nc.dram_tensor("idxB", [128, cfg.WPC * cfg.SLB // 16], i16, kind="ExternalInput")
    ldA = nc.dram_tensor("ldA", [128, cfg.WPC * cfg.NSA], bf, kind="ExternalInput")
    ldB = nc.dram_tensor("ldB", [128, cfg.WPC * cfg.NSB], bf, kind="ExternalInput")
    csrA = nc.dram_tensor("csrA", [128, cfg.WPC * 2], f16, kind="ExternalInput")
    csrB = nc.dram_tensor("csrB", [128, cfg.WPC * 2], f16, kind="ExternalInput")

    o1sh, o1ag = [], []
    coff = [0]
    for j, cs in enumerate(cfg.CHUNKS):
        coff.append(coff[-1] + cs)
        o1sh.append(nc.dram_tensor(f"o1sh{j}", [C1, cs * 128], bf))
        aspace = "Shared" if cfg.NC > 4 else "Local"
        o1ag.append(nc.dram_tensor(f"o1ag{j}", [cfg.NC, C1, cs * 128], bf,
                                   addr_space=aspace))

    def chunk_of(wi):
        j = 0
        while wi >= coff[j + 1]:
            j += 1
        return j, (wi - coff[j]) * 128

    with tile.TileContext(nc) as tc:
        with (
            tc.tile_pool(name="const", bufs=1) as cp,
            tc.tile_pool(name="meta", bufs=1) as mp,
        ):
            iota_i = cp.tile([128, 128], i16, tag="ioi")
            nc.gpsimd.iota(iota_i[:], pattern=[[1, 128]], base=0, channel_multiplier=0)
            iota_bf = cp.tile([128, 128], bf, tag="iob")
            nc.vector.tensor_copy(iota_bf[:], iota_i[:])
            iota_h = cp.tile([128, 128], f16, tag="ioh")
            nc.vector.tensor_copy(iota_h[:], iota_i[:])
            ident = cp.tile([128, 128], bf, tag="idn")
            masks.make_identity(nc, ident[:])

            idxA_t = mp.tile([128, cfg.WPC * cfg.SLA // 16], i16, tag="ixa")
            nc.sync.dma_start(idxA_t[:], idxA[:, :])
            idxB_t = mp.tile([128, cfg.WPC * cfg.SLB // 16], i16, tag="ixb")
            nc.sync.dma_start(idxB_t[:], idxB[:, :])
            ldA_t = mp.tile([128, cfg.WPC * cfg.NSA], bf, tag="lda")
            nc.sync.dma_start(ldA_t[:], ldA[:, :])
            ldB_t = mp.tile([128, cfg.WPC * cfg.NSB], bf, tag="ldb")
            nc.sync.dma_start(ldB_t[:], ldB[:, :])
            csrA_t = mp.tile([128, cfg.WPC * 2], f16, tag="csa")
            nc.sync.dma_start(csrA_t[:], csrA[:, :])
            csrB_t = mp.tile([128, cfg.WPC * 2], f16, tag="csb")
            nc.sync.dma_start(csrB_t[:], csrB[:, :])

            b1_t = mp.tile([128, C1], f32, tag="b1")
            nc.sync.dma_start(b1_t[:], b1r[:, :])
            b2_t = mp.tile([128, GOUT], f32, tag="b2")
            nc.sync.dma_start(b2_t[:], b2r[:, :])
            wm1_t = mp.tile([GOUT, MLP_HID], bf, tag="wm1")
            nc.sync.dma_start(wm1_t[:], wm1[:, :])
            wm2_t = mp.tile([MLP_HID, OUT_CLASSES], bf, tag="wm2")
            nc.sync.dma_start(wm2_t[:], wm2[:, :])
            bm1_t = mp.tile([MLP_HID, 1], f32, tag="bm1")
            nc.sync.dma_start(bm1_t[:], bm1c[:, :])
            bm2_t = mp.tile([OUT_CLASSES, 1], f32, tag="bm2")
            nc.sync.dma_start(bm2_t[:], bm2c[:, :])

            # ---------------- phase 0: layer-1 dense (replicated) ------------
            with (
                tc.tile_pool(name="p0", bufs=3) as p0,
                tc.tile_pool(name="p0w", bufs=1) as p0w,
                tc.tile_pool(name="p0ps", bufs=4, space="PSUM") as p0ps,
            ):
                w1_t = p0w.tile([IN_CH, C1 + 16], bf, tag="w1")
                nc.sync.dma_start(w1_t[:], w1[:, :])
                XB = 4
                for nt0 in range(0, cfg.NWIN, XB):
                    xc = p0.tile([IN_CH, XB * 128], bf, tag="xc")
                    nc.sync.dma_start(xc[:], xT[:, nt0 * 128 : (nt0 + XB) * 128])
                    for k in range(XB):
                        nt = nt0 + k
                        ps = p0ps.tile([128, C1 + 16], f32, tag="ps")
                        nc.tensor.matmul(ps[:], lhsT=xc[:, k * 128 : (k + 1) * 128],
                                         rhs=w1_t[:], start=True, stop=True)
                        yb = p0.tile([128, RW1], mybir.dt.uint16, tag="yb")
                        nc.vector.tensor_copy(yb[:, 0:C1].bitcast(bf), ps[:, 0:C1])
                        nc.vector.tensor_copy(
                            yb[:, C1 : C1 + 32].bitcast(f32), ps[:, C1 : C1 + 16])
                        nc.vector.memset(yb[:, C1 + 32 : RW1], 0.0)
                        nc.sync.dma_start(Y1[nt * 128 : (nt + 1) * 128, :], yb[:])

            # ---------------- shared edge phase ------------
            def edge_phase(Y, RW, C, finish_window):
                combined = (C + HEADS) * 4 <= 2048
                with (
                    tc.tile_pool(name="eg", bufs=2) as eg,
                    tc.tile_pool(name="ew", bufs=3) as ew,
                    tc.tile_pool(name="eo", bufs=2) as eo,
                    tc.tile_pool(name="psA", bufs=2, space="PSUM") as psA,
                    tc.tile_pool(name="psD", bufs=2, space="PSUM") as psD,
                    tc.tile_pool(name="psS", bufs=4, space="PSUM") as psS,
                ):
                    def watt_of(as_ap, ad_ap):
                        watt = ew.tile([128, HEADS], f32, tag="wt")
                        nc.vector.tensor_tensor(watt[:], as_ap, ad_ap, OP.add)
                        wab = ew.tile([128, HEADS], f32, tag="wb")
                        nc.scalar.activation(wab[:], watt[:], AF.Abs, scale=0.4)
                        nc.vector.scalar_tensor_tensor(
                            out=watt[:], in0=watt[:], scalar=0.6, in1=wab[:],
                            op0=OP.mult, op1=OP.add)
                        nc.scalar.activation(watt[:], watt[:], AF.Exp)
                        return watt

                    def msg_of(h_ap, watt):
                        msg = ew.tile([128, C + HEADS], bf, tag="mg")
                        nc.vector.tensor_tensor(
                            msg[:, 0:C].rearrange("p (h c) -> p h c", h=HEADS),
                            h_ap.rearrange("p (h c) -> p h c", h=HEADS),
                            watt[:].unsqueeze(2).broadcast_to([128, HEADS, C // HEADS]),
                            OP.mult)
                        nc.vector.tensor_copy(msg[:, C : C + HEADS], watt[:])
                        return msg

                    def seg_mm(ps_o, ps_den, lhsT, msg, start, stop):
                        if combined:
                            nc.tensor.matmul(ps_o[:], lhsT=lhsT, rhs=msg[:],
                                             start=start, stop=stop)
                        else:
                            nc.tensor.matmul(ps_o[:], lhsT=lhsT, rhs=msg[:, 0:C],
                                             start=start, stop=stop)
                            nc.tensor.matmul(ps_den[:], lhsT=lhsT,
                                             rhs=msg[:, C : C + HEADS],
                                             start=start, stop=stop)

                    for g in range(cfg.WPC // cfg.GRP):
                        na16 = cfg.GRP * (cfg.SLA // 16)
                        nb16 = cfg.GRP * (cfg.SLB // 16)
                        gbufA = eg.tile([128, cfg.GRP * cfg.NSA, RW], mybir.dt.uint16, tag="gA")
                        nc.gpsimd.dma_gather(
                            gbufA[:], Y[0 : cfg.ASPLIT, :],
                            idxA_t[:, g * na16 : (g + 1) * na16],
                            cfg.GRP * cfg.SLA, cfg.GRP * cfg.SLA, RW)
                        gbufB = eg.tile([128, cfg.GRP * cfg.NSB, RW], mybir.dt.uint16, tag="gB")
                        nc.gpsimd.dma_gather(
                            gbufB[:], Y[cfg.ASPLIT : cfg.NPAD, :],
                            idxB_t[:, g * nb16 : (g + 1) * nb16],
                            cfg.GRP * cfg.SLB, cfg.GRP * cfg.SLB, RW)
                        for wl in range(cfg.GRP):
                            wi = g * cfg.GRP + wl

                            def fview(gbuf, sub):
                                return gbuf[:, sub, C : C + 32].bitcast(f32)

                            adw = ew.tile([128, HEADS], f32, tag="adw")
                            nc.vector.tensor_tensor(
                                adw[:], fview(gbufA, wl * cfg.NSA)[:, 8:16],
                                fview(gbufB, wl * cfg.NSB)[:, 8:16], OP.add)
                            adw_bf = ew.tile([128, HEADS], bf, tag="adb")
                            nc.vector.tensor_copy(adw_bf[:], adw[:])

                            if combined:
                                ps_o = psA.tile([128, C + HEADS], f32, tag="po")
                                ps_den = None
                                den_ap = ps_o[:, C : C + HEADS]
                            else:
                                ps_o = psA.tile([128, C], f32, tag="po")
                                ps_den = psD.tile([128, HEADS], f32, tag="pd")
                                den_ap = ps_den[:]

                            first = True
                            for stream in (0, 1):
                                ns = cfg.NSA if stream == 0 else cfg.NSB
                                gbuf = gbufA if stream == 0 else gbufB
                                ldt = ldA_t if stream == 0 else ldB_t
                                csrt = csrA_t if stream == 0 else csrB_t
                                for t in range(ns):
                                    sub = wl * ns + t
                                    ldq = wi * ns + t
                                    last = stream == 1 and t == ns - 1
                                    M = ew.tile([128, 128], bf, tag="M")
                                    nc.vector.tensor_tensor(
                                        M[:],
                                        ldt[:, ldq : ldq + 1].broadcast_to([128, 128]),
                                        iota_bf[:], OP.is_equal)
                                    as_ap = fview(gbuf, sub)[:, 0:8]
                                    if t == 0:
                                        ad_src = fview(gbuf, sub)[:, 8:16]
                                    else:
                                        tr = t - 1
                                        ge = ew.tile([128, 128], f16, tag="ge")
                                        nc.vector.scalar_tensor_tensor(
                                            out=ge[:],
                                            in0=csrt[:, wi * 2 : wi * 2 + 1].broadcast_to([128, 128]),
                                            scalar=float(-tr * 128), in1=iota_h[:],
                                            op0=OP.add, op1=OP.is_le)
                                        lt = ew.tile([128, 128], f16, tag="lt")
                                        nc.vector.scalar_tensor_tensor(
                                            out=lt[:],
                                            in0=csrt[:, wi * 2 + 1 : wi * 2 + 2].broadcast_to([128, 128]),
                                            scalar=float(-tr * 128), in1=iota_h[:],
                                            op0=OP.add, op1=OP.is_gt)
                                        MT = ew.tile([128, 128], bf, tag="MT")
                                        nc.vector.tensor_tensor(MT[:], ge[:], lt[:], OP.mult)
                                        ps_ad = psS.tile([128, HEADS], f32, tag="sm")
                                        nc.tensor.matmul(ps_ad[:], lhsT=MT[:],
                                                         rhs=adw_bf[:],
                                                         start=True, stop=True)
                                        ad_src = ps_ad[:]
                                    watt = watt_of(as_ap, ad_src)
                                    msg = msg_of(gbuf[:, sub, 0:C].bitcast(bf), watt)
                                    seg_mm(ps_o, ps_den, M[:], msg, first, last)
                                    first = False

                            finish_window(wi, ps_o, den_ap, (ew, eo, psS))

            # ---------------- phase 1: layer-1 edge ------------
            def finish1(wi, ps_o, den_ap, pools):
                ew, eo, psS = pools
                denr = ew.tile([128, HEADS], f32, tag="dnr")
                nc.vector.reciprocal(denr[:], den_ap)
                outn = eo.tile([128, C1], f32, tag="on")
                nc.vector.tensor_tensor(
                    outn[:].rearrange("p (h c) -> p h c", h=HEADS),
                    ps_o[:, 0:C1].rearrange("p (h c) -> p h c", h=HEADS),
                    denr[:].unsqueeze(2).broadcast_to([128, HEADS, HID]),
                    OP.mult)
                nc.vector.tensor_tensor(outn[:], outn[:], b1_t[:], OP.add)
                outb = eo.tile([128, C1], bf, tag="ob")
                nc.scalar.activation(outb[:], outn[:], AF.Relu)
                j, col0 = chunk_of(wi)
                for cb in range(C1 // 128):
                    pst = psS.tile([128, 128], bf, tag="sm")
                    nc.tensor.transpose(pst[:], outb[:, cb * 128 : (cb + 1) * 128],
                                        ident[:])
                    sbt = eo.tile([128, 128], bf, tag="st")
                    nc.vector.tensor_copy(sbt[:], pst[:])
                    nc.sync.dma_start(
                        o1sh[j][cb * 128 : (cb + 1) * 128, col0 : col0 + 128], sbt[:])

            edge_phase(Y1, RW1, C1, finish1)

            if os.environ.get("GAT_NO_CC") == "1":
                # timing bisect: local copies instead of collectives (WRONG results)
                for j in range(len(cfg.CHUNKS)):
                    for r in range(cfg.NC):
                        nc.sync.dma_start(o1ag[j][r, :, :], o1sh[j][:, :])
            else:
                for j in range(len(cfg.CHUNKS)):
                    nc.gpsimd.collective_compute(
                        "AllGather", OP.bypass,
                        replica_groups=[list(range(cfg.NC))],
                        ins=[o1sh[j][:, :]], outs=[o1ag[j][:, :, :]])

            # ---------------- phase 2: layer-2 dense (replicated) ------------
            with (
                tc.tile_pool(name="p2", bufs=4) as p2,
                tc.tile_pool(name="p2w", bufs=1) as p2w,
                tc.tile_pool(name="p2ps", bufs=2, space="PSUM") as p2ps,
                tc.tile_pool(name="p2psb", bufs=2, space="PSUM") as p2psb,
            ):
                w2k0 = p2w.tile([128, C2 + 16], bf, tag="w2a")
                nc.sync.dma_start(w2k0[:], w2[0:128, :])
                w2k1 = p2w.tile([128, C2 + 16], bf, tag="w2b")
                nc.sync.dma_start(w2k1[:], w2[128:256, :])
                for nt in range(cfg.NWIN):
                    r, jj = nt // cfg.WPC, nt % cfg.WPC
                    j, col0 = chunk_of(jj)
                    l0 = p2.tile([128, 128], bf, tag="l0")
                    nc.sync.dma_start(l0[:], o1ag[j][r, 0:128, col0 : col0 + 128])
                    l1 = p2.tile([128, 128], bf, tag="l1")
                    nc.sync.dma_start(l1[:], o1ag[j][r, 128:256, col0 : col0 + 128])
                    psa = p2ps.tile([128, 512], f32, tag="pa")
                    psb = p2psb.tile([128, C2 + 16 - 512], f32, tag="pb")
                    nc.tensor.matmul(psa[:], lhsT=l0[:], rhs=w2k0[:, 0:512],
                                     start=True, stop=False)
                    nc.tensor.matmul(psb[:], lhsT=l0[:], rhs=w2k0[:, 512 : C2 + 16],
                                     start=True, stop=False)
                    nc.tensor.matmul(psa[:], lhsT=l1[:], rhs=w2k1[:, 0:512],
                                     start=False, stop=True)
                    nc.tensor.matmul(psb[:], lhsT=l1[:], rhs=w2k1[:, 512 : C2 + 16],
                                     start=False, stop=True)
                    yb = p2.tile([128, RW2], mybir.dt.uint16, tag="yb")
                    nc.vector.tensor_copy(yb[:, 0:C2].bitcast(bf), psa[:])
                    nc.vector.tensor_copy(
                        yb[:, C2 : C2 + 32].bitcast(f32), psb[:, 0:16])
                    nc.vector.memset(yb[:, C2 + 32 : RW2], 0.0)
                    nc.sync.dma_start(Y2[nt * 128 : (nt + 1) * 128, :], yb[:])

            # ---------------- phase 3: layer-2 edge + MLP ------------
            def finish2(wi, ps_o, den_ap, pools):
                ew, eo, psS = pools
                denr = ew.tile([128, HEADS], f32, tag="dnr")
                nc.vector.reciprocal(denr[:], den_ap)
                outn = eo.tile([128, C2], f32, tag="on")
                nc.vector.tensor_tensor(
                    outn[:].rearrange("p (h c) -> p h c", h=HEADS),
                    ps_o[:, 0:C2].rearrange("p (h c) -> p h c", h=HEADS),
                    denr[:].unsqueeze(2).broadcast_to([128, HEADS, GOUT]),
                    OP.mult)
                h2m = eo.tile([128, GOUT], f32, tag="h2")
                nc.vector.tensor_tensor(h2m[:], outn[:, 0:GOUT],
                                        outn[:, GOUT : 2 * GOUT], OP.add)
                for hd in range(2, HEADS):
                    nc.vector.tensor_tensor(
                        h2m[:], h2m[:], outn[:, hd * GOUT : (hd + 1) * GOUT], OP.add)
                nc.vector.scalar_tensor_tensor(
                    out=h2m[:], in0=h2m[:], scalar=1.0 / HEADS, in1=b2_t[:],
                    op0=OP.mult, op1=OP.add)
                h2b = eo.tile([128, GOUT], bf, tag="h2b")
                nc.vector.tensor_copy(h2b[:], h2m[:])
                pst = psS.tile([GOUT, 128], bf, tag="sm")
                nc.tensor.transpose(pst[:], h2b[:], ident[:])
                h2t = eo.tile([GOUT, 128], bf, tag="h2t")
                nc.vector.tensor_copy(h2t[:], pst[:])
                ps_hm = psS.tile([MLP_HID, 128], f32, tag="sm")
                nc.tensor.matmul(ps_hm[:], lhsT=wm1_t[:], rhs=h2t[:],
                                 start=True, stop=True)
                hmr = eo.tile([MLP_HID, 128], bf, tag="hmr")
                nc.scalar.activation(hmr[:], ps_hm[:], AF.Relu, bias=bm1_t[:])
                ps_po = psS.tile([OUT_CLASSES, 128], f32, tag="sm")
                nc.tensor.matmul(ps_po[:], lhsT=wm2_t[:], rhs=hmr[:],
                                 start=True, stop=True)
                outf = eo.tile([OUT_CLASSES, 128], f32, tag="of")
                nc.scalar.activation(outf[:], ps_po[:], AF.Identity, bias=bm2_t[:])
                nc.sync.dma_start(outT[:, wi * 128 : (wi + 1) * 128], outf[:])

            edge_phase(Y2, RW2, C2, finish2)

    nc.finalize()
    return nc


# ----------------------------------------------------------------------------
# host orchestration
# ----------------------------------------------------------------------------

def host_prep(x, edge_index, W1, a1_src, a1_dst, b1, W2, a2_src, a2_dst, b2,
              Wm1, bm1, Wm2, bm2, cfg):
    x = np.asarray(x, np.float32)
    ei = np.asarray(edge_index)
    lab = relabel(np.asarray(ei[1], np.int64), cfg)
    cores = build_streams(np.asarray(ei[0], np.int64),
                          np.asarray(ei[1], np.int64), lab, cfg)
    packed = [pack_core(s, cfg) for s in cores]

    W1 = np.asarray(W1, np.float32)
    W2 = np.asarray(W2, np.float32)
    w1aug = np.concatenate(
        [W1, W1 @ _blockdiag(np.asarray(a1_src, np.float32), HID),
         W1 @ _blockdiag(np.asarray(a1_dst, np.float32), HID)], 1).astype(BF16)
    w2aug = np.concatenate(
        [W2, W2 @ _blockdiag(np.asarray(a2_src, np.float32), GOUT),
         W2 @ _blockdiag(np.asarray(a2_dst, np.float32), GOUT)], 1).astype(BF16)

    xT = np.zeros((IN_CH, cfg.NPAD), BF16)
    xT[:, lab[: cfg.N]] = x.T.astype(BF16)

    common = dict(
        xT=xT, w1=w1aug, w2=w2aug,
        b1r=np.tile(np.asarray(b1, np.float32)[None, :], (128, 1)),
        b2r=np.tile(np.asarray(b2, np.float32)[None, :], (128, 1)),
        wm1=np.asarray(Wm1, np.float32).astype(BF16),
        wm2=np.asarray(Wm2, np.float32).astype(BF16),
        bm1c=np.ascontiguousarray(np.asarray(bm1, np.float32)[:, None]),
        bm2c=np.ascontiguousarray(np.asarray(bm2, np.float32)[:, None]),
    )
    in_maps = [{**common, **packed[c]} for c in range(cfg.NC)]
    return in_maps, lab


_RUNNER = {}


def _make_runner(nc, cfg):
    import jax
    from jax.sharding import Mesh, PartitionSpec
    from jax.experimental.shard_map import shard_map
    from concourse import bass2jax, mybir

    bass2jax.install_neuronx_cc_hook()
    in_names, out_names, out_avals, zero_shapes = [], [], [], []
    partition_name = nc.partition_id_tensor.name if nc.partition_id_tensor else None
    for alloc in nc.m.functions[0].allocations:
        if not isinstance(alloc, mybir.MemoryLocationSet):
            continue
        name = alloc.memorylocations[0].name
        if alloc.kind == "ExternalInput":
            if name != partition_name:
                in_names.append(name)
        elif alloc.kind == "ExternalOutput":
            sh = list(alloc.tensor_shape)
            dt = mybir.dt.np(alloc.dtype)
            out_names.append(name)
            out_avals.append(jax.core.ShapedArray(tuple(sh), dt))
            zero_shapes.append((sh, dt))
    n_params, n_outs = len(in_names), len(out_names)
    all_in = in_names + out_names + ([partition_name] if partition_name else [])
    donate = tuple(range(n_params, n_params + n_outs))

    def _body(*args):
        ops = list(args)
        if partition_name:
            ops.append(bass2jax.partition_id_tensor())
        return tuple(bass2jax._bass_exec_p.bind(
            *ops, out_avals=tuple(out_avals), in_names=tuple(all_in),
            out_names=tuple(out_names), lowering_input_output_aliases=(),
            sim_require_finite=False, sim_require_nnan=False, nc=nc))

    devices = jax.devices()[: cfg.NC]
    mesh = Mesh(np.asarray(devices), ("core",))
    specs = (PartitionSpec("core"),) * (n_params + n_outs)
    fn = jax.jit(shard_map(_body, mesh=mesh, in_specs=specs,
                           out_specs=(PartitionSpec("core"),) * n_outs,
                           check_rep=False),
                 donate_argnums=donate, keep_unused=True)
    return dict(fn=fn, in_names=in_names, out_names=out_names,
                out_avals=out_avals, zero_shapes=zero_shapes)


def _run(pack, in_maps, cfg, dev_in=None):
    import jax
    if dev_in is None:
        dev_in = {}
        for nm in pack["in_names"]:
            a = np.concatenate([np.asarray(in_maps[c][nm]) for c in range(cfg.NC)], 0)
            dev_in[nm] = jax.device_put(a)
    zeros = [jax.device_put(np.zeros((cfg.NC * s[0], *s[1:]), d))
             for s, d in pack["zero_shapes"]]
    outs = pack["fn"](*[dev_in[nm] for nm in pack["in_names"]], *zeros)
    jax.block_until_ready(outs)
    res = [
        {nm: np.asarray(outs[i]).reshape(cfg.NC, *pack["out_avals"][i].shape)[c]
         for i, nm in enumerate(pack["out_names"])}
        for c in range(cfg.NC)
    ]
    return res, dev_in, outs


LAST_EXEC_NS = None
_LAST_DEV_IN = None


def kernel(x, edge_index, W1, a1_src, a1_dst, b1, W2, a2_src, a2_dst, b2,
           Wm1, bm1, Wm2, bm2):
    global _LAST_DEV_IN
    cfg = FULL
    in_maps, lab = host_prep(x, edge_index, W1, a1_src, a1_dst, b1,
                             W2, a2_src, a2_dst, b2, Wm1, bm1, Wm2, bm2, cfg)
    if "all" not in _RUNNER:
        nc = build_nc(cfg, part="all")
        _RUNNER["all"] = _make_runner(nc, cfg)
    res, dev_in, outs = _run(_RUNNER["all"], in_maps, cfg)
    _LAST_DEV_IN = dev_in

    outT = np.concatenate([res[c]["outT"] for c in range(cfg.NC)], axis=1)
    out = outT.T[lab[: cfg.N]]
    return np.ascontiguousarray(out, np.float32)


def bench(n=5, pipeline=12):
    """Re-execute the compiled program on device-resident inputs.

    Returns (serial_walls, pipelined_per_call): serial walls include the
    axon dispatch round trip; the pipelined estimate enqueues `pipeline`
    executions and blocks once, so per-call ~= device execution time.
    Sets LAST_EXEC_NS to the pipelined estimate."""
    global LAST_EXEC_NS
    import jax
    cfg = FULL
    pack = _RUNNER["all"]
    dev_in = _LAST_DEV_IN
    args = [dev_in[nm] for nm in pack["in_names"]]
    nz = n + 2 * pipeline
    zsets = [[jax.device_put(np.zeros((cfg.NC * s[0], *s[1:]), d))
              for s, d in pack["zero_shapes"]] for _ in range(nz + 1)]
    jax.block_until_ready(zsets)
    o = pack["fn"](*args, *zsets[0])
    jax.block_until_ready(o)
    times = []
    for r in range(1, n + 1):
        t0 = time.perf_counter()
        o = pack["fn"](*args, *zsets[r])
        jax.block_until_ready(o)
        times.append(time.perf_counter() - t0)
    # pipelined: enqueue `pipeline` calls, block once; do twice, take min
    per_call = []
    for rep in range(2):
        zs = zsets[n + 1 + rep * pipeline : n + 1 + (rep + 1) * pipeline]
        t0 = time.perf_counter()
        outs = [pack["fn"](*args, *z) for z in zs]
        jax.block_until_ready(outs)
        dt = time.perf_counter() - t0
        per_call.append((dt - min(times)) / max(1, pipeline - 1))
    est = max(min(per_call), 0.0)
    LAST_EXEC_NS = int(est * 1e9)
    return times, per_call


# revision 13
# speedup vs baseline: 1.2092x; 1.2092x over previous
"""GAT (2-layer, 8 heads) + MLP on 8 Trainium2 NeuronCores — full Bass kernel.

Node-parallel layout (per sharding hint): nodes are relabeled host-side so
every 128-node "window" has balanced in-degree, then row-sharded 8 ways
(core c owns windows [c*WPC, (c+1)*WPC)). Per layer:
  dense:  Y = X @ Waug on the tensor engine (Waug folds the attention
          projections: each Y row is [h | a_src.h | a_dst.h]; the 16
          attention logits are stored f32-bitcast inside the bf16 row),
          replicated per core so the edge gather reads core-local HBM.
  edges:  MoE-style dma_gather pulls Y[src] rows for each core's incoming
          edges (dst-sorted, window-padded streams; src<32768 and >=32768
          gathered separately to fit int16 indices; labels 0 and ASPLIT
          are reserved zero-feature pad nodes so pad slots gather zeros).
          Softmax runs without max-subtraction (exp of small logits), so
          attention+aggregation collapse to one weighted scatter:
          out[n] = sum_e w_e*[h_src|1], computed as one-hot matmuls into
          PSUM per window. Each window's stream carries a leading "self
          subtile" (the window's own 128 rows, in order) in whichever
          stream its labels fall in — this supplies both the self-loop
          edges and the window's a_dst table.
  The inter-layer exchange (relu(out1) must be visible to all cores for
  layer-2's dense) is an in-kernel AllGather collective.
"""
import os
import sys
import time

for _p in ("/opt/trn_rl_repo",):
    if _p not in sys.path:
        sys.path.append(_p)

import numpy as np
import ml_dtypes

# model dims (fixed by the problem)
IN_CH, HID, GOUT, HEADS = 128, 32, 64, 8
MLP_HID, OUT_CLASSES, NEG = 64, 2, 0.2
C1, C2 = HEADS * HID, HEADS * GOUT          # 256, 512
RW1, RW2 = 384, 640                          # padded Y row widths (bf16)

BF16 = ml_dtypes.bfloat16


class Cfg:
    def __init__(self, N, NPAD, NC, WPC, ASPLIT, CAPA, CAPB, GRP, CHUNKS):
        self.N, self.NPAD, self.NC, self.WPC = N, NPAD, NC, WPC
        self.ASPLIT, self.CAPA, self.CAPB = ASPLIT, CAPA, CAPB
        self.NSA, self.NSB = CAPA // 128 + 1, CAPB // 128 + 1  # incl self subtile
        self.SLA, self.SLB = CAPA + 128, CAPB + 128            # slots per window
        self.GRP, self.CHUNKS = GRP, tuple(CHUNKS)
        self.NWIN = NPAD // 128
        assert NPAD == NC * WPC * 128 and WPC % GRP == 0 and sum(CHUNKS) == WPC
        assert ASPLIT % 128 == 0


FULL = Cfg(N=50000, NPAD=50176, NC=8, WPC=49, ASPLIT=32768,
           CAPA=768, CAPB=512, GRP=1, CHUNKS=(13, 12, 12, 12))


# ----------------------------------------------------------------------------
# host-side graph prep
# ----------------------------------------------------------------------------

def relabel(dst_noloop, cfg):
    deg = np.bincount(dst_noloop, minlength=cfg.N)
    deg_all = np.concatenate([deg, np.zeros(cfg.NPAD - cfg.N, np.int64)])
    order = np.argsort(-deg_all, kind="stable")
    win_of = np.empty(cfg.NPAD, np.int64)
    fwd = np.arange(cfg.NWIN)
    rev = fwd[::-1]
    nb = cfg.NPAD // cfg.NWIN
    for b in range(nb):
        idxs = order[b * cfg.NWIN : (b + 1) * cfg.NWIN]
        win_of[idxs] = fwd if b % 2 == 0 else rev
    lab = np.empty(cfg.NPAD, np.int64)
    perm = np.lexsort((np.arange(cfg.NPAD), win_of))
    lab[perm] = np.arange(cfg.NPAD)
    # reserve labels 0 and ASPLIT for zero-feature pad nodes (so index-0
    # pad slots in each stream gather zero rows)
    inv = np.argsort(lab)
    for want, padn in ((0, cfg.N), (cfg.ASPLIT, cfg.N + 1)):
        holder = inv[want]
        if holder >= cfg.N:
            continue
        lab[holder], lab[padn] = lab[padn], lab[holder]
        inv = np.argsort(lab)
    return lab


def build_streams(src, dst, lab, cfg):
    """src/dst: random edges only. Self-loops become per-window self
    subtiles (slots [0:128] of the window's A or B stream)."""
    sl = lab[src]
    dl = lab[dst]
    order = np.argsort(dl, kind="stable")
    sl, dl = sl[order], dl[order]
    w = (dl >> 7).astype(np.int64)
    ld = (dl & 127).astype(np.int64)
    isA = sl < cfg.ASPLIT
    wstarts = np.searchsorted(w, np.arange(cfg.NWIN + 1))
    cores = []
    ar = np.arange(128)
    for c in range(cfg.NC):
        idxA = np.zeros((cfg.WPC, cfg.SLA), np.int16)
        idxB = np.zeros((cfg.WPC, cfg.SLB), np.int16)
        ldA = np.full((cfg.WPC, cfg.SLA), -1.0, np.float32)
        ldB = np.full((cfg.WPC, cfg.SLB), -1.0, np.float32)
        csrA = np.zeros((cfg.WPC, 128, 2), np.float32)
        csrB = np.zeros((cfg.WPC, 128, 2), np.float32)
        for wi in range(cfg.WPC):
            gw = c * cfg.WPC + wi
            base = gw * 128
            if base < cfg.ASPLIT:
                idxA[wi, 0:128] = (base + ar).astype(np.int16)
                ldA[wi, 0:128] = ar
            else:
                idxB[wi, 0:128] = (base - cfg.ASPLIT + ar).astype(np.int16)
                ldB[wi, 0:128] = ar
            s0, s1 = wstarts[gw], wstarts[gw + 1]
            wsl, wld, wA = sl[s0:s1], ld[s0:s1], isA[s0:s1]
            for stream, cap, idx_o, ld_o, csr_o in (
                (True, cfg.CAPA, idxA, ldA, csrA),
                (False, cfg.CAPB, idxB, ldB, csrB),
            ):
                mm = wA == stream
                s_ids = wsl[mm] if stream else wsl[mm] - cfg.ASPLIT
                l_ids = wld[mm]
                cnt = len(s_ids)
                assert cnt <= cap, f"stream overflow: core {c} win {wi} {cnt}>{cap}"
                idx_o[wi, 128 : 128 + cnt] = s_ids.astype(np.int16)
                ld_o[wi, 128 : 128 + cnt] = l_ids
                csr_o[wi, :, 0] = np.searchsorted(l_ids, ar, side="left")
                csr_o[wi, :, 1] = np.searchsorted(l_ids, ar, side="right")
        cores.append(dict(idxA=idxA, idxB=idxB, ldA=ldA, ldB=ldB,
                          csrA=csrA, csrB=csrB))
    return cores


def pack_core(st, cfg):
    def wrap_idx(a, slots):
        w16 = (a.reshape(cfg.WPC, slots // 16, 16).transpose(2, 0, 1)
               .reshape(16, cfg.WPC * (slots // 16)))
        return np.tile(w16, (8, 1)).copy()

    def wrap_ld(a, ns):
        return (a.reshape(cfg.WPC, ns, 128).transpose(2, 0, 1)
                .reshape(128, cfg.WPC * ns)).astype(BF16).copy()

    def wrap_csr(a):
        return (a.transpose(1, 0, 2).reshape(128, cfg.WPC * 2)
                .astype(np.float16).copy())

    return dict(
        idxA=wrap_idx(st["idxA"], cfg.SLA),
        idxB=wrap_idx(st["idxB"], cfg.SLB),
        ldA=wrap_ld(st["ldA"], cfg.NSA),
        ldB=wrap_ld(st["ldB"], cfg.NSB),
        csrA=wrap_csr(st["csrA"]),
        csrB=wrap_csr(st["csrB"]),
    )


def _blockdiag(a, ch):
    B = np.zeros((HEADS * ch, HEADS), np.float32)
    for hd in range(HEADS):
        B[hd * ch : (hd + 1) * ch, hd] = a[hd]
    return B


# ----------------------------------------------------------------------------
# device program
# ----------------------------------------------------------------------------

def build_nc(cfg, part="all"):
    from concourse import bass, mybir
    import concourse.bacc as bacc
    import concourse.tile as tile
    import concourse.masks as masks

    bf = mybir.dt.bfloat16
    f32 = mybir.dt.float32
    f16 = mybir.dt.float16
    i16 = mybir.dt.int16
    AF = mybir.ActivationFunctionType
    OP = mybir.AluOpType

    nc = bacc.Bacc(num_devices=cfg.NC)
    SH = cfg.WPC * 128

    xT = nc.dram_tensor("xT", [IN_CH, cfg.NPAD], bf, kind="ExternalInput")
    w1 = nc.dram_tensor("w1", [IN_CH, C1 + 16], bf, kind="ExternalInput")
    b1r = nc.dram_tensor("b1r", [128, C1], f32, kind="ExternalInput")
    Y1 = nc.dram_tensor("Y1", [cfg.NPAD, RW1], mybir.dt.uint16)
    w2 = nc.dram_tensor("w2", [C1, C2 + 16], bf, kind="ExternalInput")
    b2r = nc.dram_tensor("b2r", [128, GOUT], f32, kind="ExternalInput")
    wm1 = nc.dram_tensor("wm1", [GOUT, MLP_HID], bf, kind="ExternalInput")
    wm2 = nc.dram_tensor("wm2", [MLP_HID, OUT_CLASSES], bf, kind="ExternalInput")
    bm1c = nc.dram_tensor("bm1c", [MLP_HID, 1], f32, kind="ExternalInput")
    bm2c = nc.dram_tensor("bm2c", [OUT_CLASSES, 1], f32, kind="ExternalInput")
    Y2 = nc.dram_tensor("Y2", [cfg.NPAD, RW2], mybir.dt.uint16)
    outT = nc.dram_tensor("outT", [OUT_CLASSES, SH], f32, kind="ExternalOutput")

    idxA = nc.dram_tensor("idxA", [128, cfg.WPC * cfg.SLA // 16], i16, kind="ExternalInput")
    idxB = nc.dram_tensor("idxB", [128, cfg.WPC * cfg.SLB // 16], i16, kind="ExternalInput")
    ldA = nc.dram_tensor("ldA", [128, cfg.WPC * cfg.NSA], bf, kind="ExternalInput")
    ldB = nc.dram_tensor("ldB", [128, cfg.WPC * cfg.NSB], bf, kind="ExternalInput")
    csrA = nc.dram_tensor("csrA", [128, cfg.WPC * 2], f16, kind="ExternalInput")
    csrB = nc.dram_tensor("csrB", [128, cfg.WPC * 2], f16, kind="ExternalInput")

    o1sh, o1ag = [], []
    coff = [0]
    for j, cs in enumerate(cfg.CHUNKS):
        coff.append(coff[-1] + cs)
        o1sh.append(nc.dram_tensor(f"o1sh{j}", [C1, cs * 128], bf))
        aspace = "Shared" if cfg.NC > 4 else "Local"
        o1ag.append(nc.dram_tensor(f"o1ag{j}", [cfg.NC, C1, cs * 128], bf,
                                   addr_space=aspace))

    def chunk_of(wi):
        j = 0
        while wi >= coff[j + 1]:
            j += 1
        return j, (wi - coff[j]) * 128

    PH = os.environ.get("GAT_PHASES", "all")
    with tile.TileContext(nc) as tc:
        with (
            tc.tile_pool(name="const", bufs=1) as cp,
            tc.tile_pool(name="meta", bufs=1) as mp,
        ):
            iota_i = cp.tile([128, 128], i16, tag="ioi")
            nc.gpsimd.iota(iota_i[:], pattern=[[1, 128]], base=0, channel_multiplier=0)
            iota_bf = cp.tile([128, 128], bf, tag="iob")
            nc.vector.tensor_copy(iota_bf[:], iota_i[:])
            iota_h = cp.tile([128, 128], f16, tag="ioh")
            nc.vector.tensor_copy(iota_h[:], iota_i[:])
            ident = cp.tile([128, 128], bf, tag="idn")
            masks.make_identity(nc, ident[:])

            idxA_t = mp.tile([128, cfg.WPC * cfg.SLA // 16], i16, tag="ixa")
            nc.sync.dma_start(idxA_t[:], idxA[:, :])
            idxB_t = mp.tile([128, cfg.WPC * cfg.SLB // 16], i16, tag="ixb")
            nc.sync.dma_start(idxB_t[:], idxB[:, :])
            ldA_t = mp.tile([128, cfg.WPC * cfg.NSA], bf, tag="lda")
            nc.sync.dma_start(ldA_t[:], ldA[:, :])
            ldB_t = mp.tile([128, cfg.WPC * cfg.NSB], bf, tag="ldb")
            nc.sync.dma_start(ldB_t[:], ldB[:, :])
            csrA_t = mp.tile([128, cfg.WPC * 2], f16, tag="csa")
            nc.sync.dma_start(csrA_t[:], csrA[:, :])
            csrB_t = mp.tile([128, cfg.WPC * 2], f16, tag="csb")
            nc.sync.dma_start(csrB_t[:], csrB[:, :])

            b1_t = mp.tile([128, C1], f32, tag="b1")
            nc.sync.dma_start(b1_t[:], b1r[:, :])
            b2_t = mp.tile([128, GOUT], f32, tag="b2")
            nc.sync.dma_start(b2_t[:], b2r[:, :])
            wm1_t = mp.tile([GOUT, MLP_HID], bf, tag="wm1")
            nc.sync.dma_start(wm1_t[:], wm1[:, :])
            wm2_t = mp.tile([MLP_HID, OUT_CLASSES], bf, tag="wm2")
            nc.sync.dma_start(wm2_t[:], wm2[:, :])
            bm1_t = mp.tile([MLP_HID, 1], f32, tag="bm1")
            nc.sync.dma_start(bm1_t[:], bm1c[:, :])
            bm2_t = mp.tile([OUT_CLASSES, 1], f32, tag="bm2")
            nc.sync.dma_start(bm2_t[:], bm2c[:, :])

            # ---------------- phase 0: layer-1 dense (replicated) ------------
            with (
                tc.tile_pool(name="p0", bufs=3) as p0,
                tc.tile_pool(name="p0w", bufs=1) as p0w,
                tc.tile_pool(name="p0ps", bufs=4, space="PSUM") as p0ps,
            ):
                w1_t = p0w.tile([IN_CH, C1 + 16], bf, tag="w1")
                nc.sync.dma_start(w1_t[:], w1[:, :])
                XB = 4
                for nt0 in range(0, cfg.NWIN, XB):
                    xc = p0.tile([IN_CH, XB * 128], bf, tag="xc")
                    nc.sync.dma_start(xc[:], xT[:, nt0 * 128 : (nt0 + XB) * 128])
                    for k in range(XB):
                        nt = nt0 + k
                        ps = p0ps.tile([128, C1 + 16], f32, tag="ps")
                        nc.tensor.matmul(ps[:], lhsT=xc[:, k * 128 : (k + 1) * 128],
                                         rhs=w1_t[:], start=True, stop=True)
                        yb = p0.tile([128, RW1], mybir.dt.uint16, tag="yb")
                        nc.vector.tensor_copy(yb[:, 0:C1].bitcast(bf), ps[:, 0:C1])
                        nc.vector.tensor_copy(
                            yb[:, C1 : C1 + 32].bitcast(f32), ps[:, C1 : C1 + 16])
                        nc.vector.memset(yb[:, C1 + 32 : RW1], 0.0)
                        nc.sync.dma_start(Y1[nt * 128 : (nt + 1) * 128, :], yb[:])

            def probe(src_ap, n_u16):
                # tiny consumer so bacc DCE keeps earlier phases alive
                with tc.tile_pool(name="pr", bufs=1) as prp:
                    pt = prp.tile([2, n_u16], mybir.dt.uint16, tag="pr")
                    nc.sync.dma_start(pt[:], src_ap)
                    nc.sync.dma_start(
                        outT[0:2, 0 : n_u16 // 2].bitcast(mybir.dt.uint16), pt[:])

            # ---------------- shared edge phase ------------
            def gather_only(Y, RW):
                with tc.tile_pool(name="eg", bufs=2) as eg:
                    for g in range(cfg.WPC // cfg.GRP):
                        na16 = cfg.GRP * (cfg.SLA // 16)
                        nb16 = cfg.GRP * (cfg.SLB // 16)
                        gbufA = eg.tile([128, cfg.GRP * cfg.NSA, RW], mybir.dt.uint16, tag="gA")
                        nc.gpsimd.dma_gather(
                            gbufA[:], Y[0 : cfg.ASPLIT, :],
                            idxA_t[:, g * na16 : (g + 1) * na16],
                            cfg.GRP * cfg.SLA, cfg.GRP * cfg.SLA, RW)
                        gbufB = eg.tile([128, cfg.GRP * cfg.NSB, RW], mybir.dt.uint16, tag="gB")
                        nc.gpsimd.dma_gather(
                            gbufB[:], Y[cfg.ASPLIT : cfg.NPAD, :],
                            idxB_t[:, g * nb16 : (g + 1) * nb16],
                            cfg.GRP * cfg.SLB, cfg.GRP * cfg.SLB, RW)
                        nc.vector.tensor_tensor(
                            gbufA[:, 0, 0:64], gbufA[:, 0, 0:64], gbufB[:, 0, 0:64],
                            OP.bitwise_or)
                    probe(gbufA[:, 0, 0:256], 256)

            def edge_phase(Y, RW, C, finish_window):
                combined = (C + HEADS) * 4 <= 2048
                with (
                    tc.tile_pool(name="eg", bufs=2) as eg,
                    tc.tile_pool(name="ew", bufs=3) as ew,
                    tc.tile_pool(name="eo", bufs=2) as eo,
                    tc.tile_pool(name="psA", bufs=2, space="PSUM") as psA,
                    tc.tile_pool(name="psD", bufs=2, space="PSUM") as psD,
                    tc.tile_pool(name="psS", bufs=4, space="PSUM") as psS,
                ):
                    def watt_of(as_ap, ad_ap):
                        watt = ew.tile([128, HEADS], f32, tag="wt")
                        nc.vector.tensor_tensor(watt[:], as_ap, ad_ap, OP.add)
                        wab = ew.tile([128, HEADS], f32, tag="wb")
                        nc.scalar.activation(wab[:], watt[:], AF.Abs, scale=0.4)
                        nc.vector.scalar_tensor_tensor(
                            out=watt[:], in0=watt[:], scalar=0.6, in1=wab[:],
                            op0=OP.mult, op1=OP.add)
                        nc.scalar.activation(watt[:], watt[:], AF.Exp)
                        return watt

                    def msg_of(h_ap, watt):
                        msg = ew.tile([128, C + HEADS], bf, tag="mg")
                        nc.vector.tensor_tensor(
                            msg[:, 0:C].rearrange("p (h c) -> p h c", h=HEADS),
                            h_ap.rearrange("p (h c) -> p h c", h=HEADS),
                            watt[:].unsqueeze(2).broadcast_to([128, HEADS, C // HEADS]),
                            OP.mult)
                        nc.vector.tensor_copy(msg[:, C : C + HEADS], watt[:])
                        return msg

                    def seg_mm(ps_o, ps_den, lhsT, msg, start, stop):
                        if combined:
                            nc.tensor.matmul(ps_o[:], lhsT=lhsT, rhs=msg[:],
                                             start=start, stop=stop)
                        else:
                            nc.tensor.matmul(ps_o[:], lhsT=lhsT, rhs=msg[:, 0:C],
                                             start=start, stop=stop)
                            nc.tensor.matmul(ps_den[:], lhsT=lhsT,
                                             rhs=msg[:, C : C + HEADS],
                                             start=start, stop=stop)

                    for g in range(cfg.WPC // cfg.GRP):
                        na16 = cfg.GRP * (cfg.SLA // 16)
                        nb16 = cfg.GRP * (cfg.SLB // 16)
                        gbufA = eg.tile([128, cfg.GRP * cfg.NSA, RW], mybir.dt.uint16, tag="gA")
                        nc.gpsimd.dma_gather(
                            gbufA[:], Y[0 : cfg.ASPLIT, :],
                            idxA_t[:, g * na16 : (g + 1) * na16],
                            cfg.GRP * cfg.SLA, cfg.GRP * cfg.SLA, RW)
                        gbufB = eg.tile([128, cfg.GRP * cfg.NSB, RW], mybir.dt.uint16, tag="gB")
                        nc.gpsimd.dma_gather(
                            gbufB[:], Y[cfg.ASPLIT : cfg.NPAD, :],
                            idxB_t[:, g * nb16 : (g + 1) * nb16],
                            cfg.GRP * cfg.SLB, cfg.GRP * cfg.SLB, RW)
                        for wl in range(cfg.GRP):
                            wi = g * cfg.GRP + wl

                            def fview(gbuf, sub):
                                return gbuf[:, sub, C : C + 32].bitcast(f32)

                            adw = ew.tile([128, HEADS], f32, tag="adw")
                            nc.vector.tensor_tensor(
                                adw[:], fview(gbufA, wl * cfg.NSA)[:, 8:16],
                                fview(gbufB, wl * cfg.NSB)[:, 8:16], OP.add)
                            adw_bf = ew.tile([128, HEADS], bf, tag="adb")
                            nc.vector.tensor_copy(adw_bf[:], adw[:])

                            if combined:
                                ps_o = psA.tile([128, C + HEADS], f32, tag="po")
                                ps_den = None
                                den_ap = ps_o[:, C : C + HEADS]
                            else:
                                ps_o = psA.tile([128, C], f32, tag="po")
                                ps_den = psD.tile([128, HEADS], f32, tag="pd")
                                den_ap = ps_den[:]

                            first = True
                            for stream in (0, 1):
                                ns = cfg.NSA if stream == 0 else cfg.NSB
                                gbuf = gbufA if stream == 0 else gbufB
                                ldt = ldA_t if stream == 0 else ldB_t
                                csrt = csrA_t if stream == 0 else csrB_t
                                for t in range(ns):
                                    sub = wl * ns + t
                                    ldq = wi * ns + t
                                    last = stream == 1 and t == ns - 1
                                    M = ew.tile([128, 128], bf, tag="M")
                                    nc.vector.tensor_tensor(
                                        M[:],
                                        ldt[:, ldq : ldq + 1].broadcast_to([128, 128]),
                                        iota_bf[:], OP.is_equal)
                                    as_ap = fview(gbuf, sub)[:, 0:8]
                                    if t == 0:
                                        ad_src = fview(gbuf, sub)[:, 8:16]
                                    else:
                                        tr = t - 1
                                        ge = ew.tile([128, 128], f16, tag="ge")
                                        nc.vector.scalar_tensor_tensor(
                                            out=ge[:],
                                            in0=csrt[:, wi * 2 : wi * 2 + 1].broadcast_to([128, 128]),
                                            scalar=float(-tr * 128), in1=iota_h[:],
                                            op0=OP.add, op1=OP.is_le)
                                        lt = ew.tile([128, 128], f16, tag="lt")
                                        nc.vector.scalar_tensor_tensor(
                                            out=lt[:],
                                            in0=csrt[:, wi * 2 + 1 : wi * 2 + 2].broadcast_to([128, 128]),
                                            scalar=float(-tr * 128), in1=iota_h[:],
                                            op0=OP.add, op1=OP.is_gt)
                                        MT = ew.tile([128, 128], bf, tag="MT")
                                        nc.vector.tensor_tensor(MT[:], ge[:], lt[:], OP.mult)
                                        ps_ad = psS.tile([128, HEADS], f32, tag="sm")
                                        nc.tensor.matmul(ps_ad[:], lhsT=MT[:],
                                                         rhs=adw_bf[:],
                                                         start=True, stop=True)
                                        ad_src = ps_ad[:]
                                    watt = watt_of(as_ap, ad_src)
                                    msg = msg_of(gbuf[:, sub, 0:C].bitcast(bf), watt)
                                    seg_mm(ps_o, ps_den, M[:], msg, first, last)
                                    first = False

                            finish_window(wi, ps_o, den_ap, (ew, eo, psS))

            # ---------------- phase 1: layer-1 edge ------------
            def finish1(wi, ps_o, den_ap, pools):
                ew, eo, psS = pools
                denr = ew.tile([128, HEADS], f32, tag="dnr")
                nc.vector.reciprocal(denr[:], den_ap)
                outn = eo.tile([128, C1], f32, tag="on")
                nc.vector.tensor_tensor(
                    outn[:].rearrange("p (h c) -> p h c", h=HEADS),
                    ps_o[:, 0:C1].rearrange("p (h c) -> p h c", h=HEADS),
                    denr[:].unsqueeze(2).broadcast_to([128, HEADS, HID]),
                    OP.mult)
                nc.vector.tensor_tensor(outn[:], outn[:], b1_t[:], OP.add)
                outb = eo.tile([128, C1], bf, tag="ob")
                nc.scalar.activation(outb[:], outn[:], AF.Relu)
                j, col0 = chunk_of(wi)
                for cb in range(C1 // 128):
                    pst = psS.tile([128, 128], bf, tag="sm")
                    nc.tensor.transpose(pst[:], outb[:, cb * 128 : (cb + 1) * 128],
                                        ident[:])
                    sbt = eo.tile([128, 128], bf, tag="st")
                    nc.vector.tensor_copy(sbt[:], pst[:])
                    nc.sync.dma_start(
                        o1sh[j][cb * 128 : (cb + 1) * 128, col0 : col0 + 128], sbt[:])

            if PH == "0":
                probe(Y1[0:2, 0:256], 256)
            if PH == "g":
                gather_only(Y1, RW1)
            if PH in ("01", "012", "all"):
                edge_phase(Y1, RW1, C1, finish1)
            if PH == "01":
                probe(o1sh[0][0:2, 0:128], 128)
            if PH in ("012", "all") and os.environ.get("GAT_NO_CC") == "1":
                # timing bisect: local copies instead of collectives (WRONG results)
                for j in range(len(cfg.CHUNKS)):
                    for r in range(cfg.NC):
                        nc.sync.dma_start(o1ag[j][r, :, :], o1sh[j][:, :])
            elif PH in ("012", "all"):
                for j in range(len(cfg.CHUNKS)):
                    nc.gpsimd.collective_compute(
                        "AllGather", OP.bypass,
                        replica_groups=[list(range(cfg.NC))],
                        ins=[o1sh[j][:, :]], outs=[o1ag[j][:, :, :]])

            # ---------------- phase 2: layer-2 dense (replicated) ------------
            if PH in ("012", "all"):
              with (
                tc.tile_pool(name="p2", bufs=4) as p2,
                tc.tile_pool(name="p2w", bufs=1) as p2w,
                tc.tile_pool(name="p2ps", bufs=2, space="PSUM") as p2ps,
                tc.tile_pool(name="p2psb", bufs=2, space="PSUM") as p2psb,
            ):
                w2k0 = p2w.tile([128, C2 + 16], bf, tag="w2a")
                nc.sync.dma_start(w2k0[:], w2[0:128, :])
                w2k1 = p2w.tile([128, C2 + 16], bf, tag="w2b")
                nc.sync.dma_start(w2k1[:], w2[128:256, :])
                for nt in range(cfg.NWIN):
                    r, jj = nt // cfg.WPC, nt % cfg.WPC
                    j, col0 = chunk_of(jj)
                    l0 = p2.tile([128, 128], bf, tag="l0")
                    nc.sync.dma_start(l0[:], o1ag[j][r, 0:128, col0 : col0 + 128])
                    l1 = p2.tile([128, 128], bf, tag="l1")
                    nc.sync.dma_start(l1[:], o1ag[j][r, 128:256, col0 : col0 + 128])
                    psa = p2ps.tile([128, 512], f32, tag="pa")
                    psb = p2psb.tile([128, C2 + 16 - 512], f32, tag="pb")
                    nc.tensor.matmul(psa[:], lhsT=l0[:], rhs=w2k0[:, 0:512],
                                     start=True, stop=False)
                    nc.tensor.matmul(psb[:], lhsT=l0[:], rhs=w2k0[:, 512 : C2 + 16],
                                     start=True, stop=False)
                    nc.tensor.matmul(psa[:], lhsT=l1[:], rhs=w2k1[:, 0:512],
                                     start=False, stop=True)
                    nc.tensor.matmul(psb[:], lhsT=l1[:], rhs=w2k1[:, 512 : C2 + 16],
                                     start=False, stop=True)
                    yb = p2.tile([128, RW2], mybir.dt.uint16, tag="yb")
                    nc.vector.tensor_copy(yb[:, 0:C2].bitcast(bf), psa[:])
                    nc.vector.tensor_copy(
                        yb[:, C2 : C2 + 32].bitcast(f32), psb[:, 0:16])
                    nc.vector.memset(yb[:, C2 + 32 : RW2], 0.0)
                    nc.sync.dma_start(Y2[nt * 128 : (nt + 1) * 128, :], yb[:])
            if PH == "012":
                probe(Y2[0:2, 0:256], 256)

            # ---------------- phase 3: layer-2 edge + MLP ------------
            def finish2(wi, ps_o, den_ap, pools):
                ew, eo, psS = pools
                denr = ew.tile([128, HEADS], f32, tag="dnr")
                nc.vector.reciprocal(denr[:], den_ap)
                outn = eo.tile([128, C2], f32, tag="on")
                nc.vector.tensor_tensor(
                    outn[:].rearrange("p (h c) -> p h c", h=HEADS),
                    ps_o[:, 0:C2].rearrange("p (h c) -> p h c", h=HEADS),
                    denr[:].unsqueeze(2).broadcast_to([128, HEADS, GOUT]),
                    OP.mult)
                h2m = eo.tile([128, GOUT], f32, tag="h2")
                nc.vector.tensor_tensor(h2m[:], outn[:, 0:GOUT],
                                        outn[:, GOUT : 2 * GOUT], OP.add)
                for hd in range(2, HEADS):
                    nc.vector.tensor_tensor(
                        h2m[:], h2m[:], outn[:, hd * GOUT : (hd + 1) * GOUT], OP.add)
                nc.vector.scalar_tensor_tensor(
                    out=h2m[:], in0=h2m[:], scalar=1.0 / HEADS, in1=b2_t[:],
                    op0=OP.mult, op1=OP.add)
                h2b = eo.tile([128, GOUT], bf, tag="h2b")
                nc.vector.tensor_copy(h2b[:], h2m[:])
                pst = psS.tile([GOUT, 128], bf, tag="sm")
                nc.tensor.transpose(pst[:], h2b[:], ident[:])
                h2t = eo.tile([GOUT, 128], bf, tag="h2t")
                nc.vector.tensor_copy(h2t[:], pst[:])
                ps_hm = psS.tile([MLP_HID, 128], f32, tag="sm")
                nc.tensor.matmul(ps_hm[:], lhsT=wm1_t[:], rhs=h2t[:],
                                 start=True, stop=True)
                hmr = eo.tile([MLP_HID, 128], bf, tag="hmr")
                nc.scalar.activation(hmr[:], ps_hm[:], AF.Relu, bias=bm1_t[:])
                ps_po = psS.tile([OUT_CLASSES, 128], f32, tag="sm")
                nc.tensor.matmul(ps_po[:], lhsT=wm2_t[:], rhs=hmr[:],
                                 start=True, stop=True)
                outf = eo.tile([OUT_CLASSES, 128], f32, tag="of")
                nc.scalar.activation(outf[:], ps_po[:], AF.Identity, bias=bm2_t[:])
                nc.sync.dma_start(outT[:, wi * 128 : (wi + 1) * 128], outf[:])

            if PH == "all":
                edge_phase(Y2, RW2, C2, finish2)

    nc.finalize()
    return nc


# ----------------------------------------------------------------------------
# host orchestration
# ----------------------------------------------------------------------------

def host_prep(x, edge_index, W1, a1_src, a1_dst, b1, W2, a2_src, a2_dst, b2,
              Wm1, bm1, Wm2, bm2, cfg):
    x = np.asarray(x, np.float32)
    ei = np.asarray(edge_index)
    lab = relabel(np.asarray(ei[1], np.int64), cfg)
    cores = build_streams(np.asarray(ei[0], np.int64),
                          np.asarray(ei[1], np.int64), lab, cfg)
    packed = [pack_core(s, cfg) for s in cores]

    W1 = np.asarray(W1, np.float32)
    W2 = np.asarray(W2, np.float32)
    w1aug = np.concatenate(
        [W1, W1 @ _blockdiag(np.asarray(a1_src, np.float32), HID),
         W1 @ _blockdiag(np.asarray(a1_dst, np.float32), HID)], 1).astype(BF16)
    w2aug = np.concatenate(
        [W2, W2 @ _blockdiag(np.asarray(a2_src, np.float32), GOUT),
         W2 @ _blockdiag(np.asarray(a2_dst, np.float32), GOUT)], 1).astype(BF16)

    xT = np.zeros((IN_CH, cfg.NPAD), BF16)
    xT[:, lab[: cfg.N]] = x.T.astype(BF16)

    common = dict(
        xT=xT, w1=w1aug, w2=w2aug,
        b1r=np.tile(np.asarray(b1, np.float32)[None, :], (128, 1)),
        b2r=np.tile(np.asarray(b2, np.float32)[None, :], (128, 1)),
        wm1=np.asarray(Wm1, np.float32).astype(BF16),
        wm2=np.asarray(Wm2, np.float32).astype(BF16),
        bm1c=np.ascontiguousarray(np.asarray(bm1, np.float32)[:, None]),
        bm2c=np.ascontiguousarray(np.asarray(bm2, np.float32)[:, None]),
    )
    in_maps = [{**common, **packed[c]} for c in range(cfg.NC)]
    return in_maps, lab


_RUNNER = {}


def _make_runner(nc, cfg):
    import jax
    from jax.sharding import Mesh, PartitionSpec
    from jax.experimental.shard_map import shard_map
    from concourse import bass2jax, mybir

    bass2jax.install_neuronx_cc_hook()
    in_names, out_names, out_avals, zero_shapes = [], [], [], []
    partition_name = nc.partition_id_tensor.name if nc.partition_id_tensor else None
    for alloc in nc.m.functions[0].allocations:
        if not isinstance(alloc, mybir.MemoryLocationSet):
            continue
        name = alloc.memorylocations[0].name
        if alloc.kind == "ExternalInput":
            if name != partition_name:
                in_names.append(name)
        elif alloc.kind == "ExternalOutput":
            sh = list(alloc.tensor_shape)
            dt = mybir.dt.np(alloc.dtype)
            out_names.append(name)
            out_avals.append(jax.core.ShapedArray(tuple(sh), dt))
            zero_shapes.append((sh, dt))
    n_params, n_outs = len(in_names), len(out_names)
    all_in = in_names + out_names + ([partition_name] if partition_name else [])
    donate = tuple(range(n_params, n_params + n_outs))

    def _body(*args):
        ops = list(args)
        if partition_name:
            ops.append(bass2jax.partition_id_tensor())
        return tuple(bass2jax._bass_exec_p.bind(
            *ops, out_avals=tuple(out_avals), in_names=tuple(all_in),
            out_names=tuple(out_names), lowering_input_output_aliases=(),
            sim_require_finite=False, sim_require_nnan=False, nc=nc))

    devices = jax.devices()[: cfg.NC]
    mesh = Mesh(np.asarray(devices), ("core",))
    specs = (PartitionSpec("core"),) * (n_params + n_outs)
    fn = jax.jit(shard_map(_body, mesh=mesh, in_specs=specs,
                           out_specs=(PartitionSpec("core"),) * n_outs,
                           check_rep=False),
                 donate_argnums=donate, keep_unused=True)
    return dict(fn=fn, in_names=in_names, out_names=out_names,
                out_avals=out_avals, zero_shapes=zero_shapes)


def _run(pack, in_maps, cfg, dev_in=None):
    import jax
    if dev_in is None:
        dev_in = {}
        for nm in pack["in_names"]:
            a = np.concatenate([np.asarray(in_maps[c][nm]) for c in range(cfg.NC)], 0)
            dev_in[nm] = jax.device_put(a)
    zeros = [jax.device_put(np.zeros((cfg.NC * s[0], *s[1:]), d))
             for s, d in pack["zero_shapes"]]
    outs = pack["fn"](*[dev_in[nm] for nm in pack["in_names"]], *zeros)
    jax.block_until_ready(outs)
    res = [
        {nm: np.asarray(outs[i]).reshape(cfg.NC, *pack["out_avals"][i].shape)[c]
         for i, nm in enumerate(pack["out_names"])}
        for c in range(cfg.NC)
    ]
    return res, dev_in, outs


LAST_EXEC_NS = None
_LAST_DEV_IN = None


def kernel(x, edge_index, W1, a1_src, a1_dst, b1, W2, a2_src, a2_dst, b2,
           Wm1, bm1, Wm2, bm2):
    global _LAST_DEV_IN
    cfg = FULL
    tlog = [("start", time.time())]
    in_maps, lab = host_prep(x, edge_index, W1, a1_src, a1_dst, b1,
                             W2, a2_src, a2_dst, b2, Wm1, bm1, Wm2, bm2, cfg)
    tlog.append(("host_prep", time.time()))
    if "all" not in _RUNNER:
        nc = build_nc(cfg, part="all")
        tlog.append(("build_nc", time.time()))
        _RUNNER["all"] = _make_runner(nc, cfg)
        tlog.append(("make_runner", time.time()))
    res, dev_in, outs = _run(_RUNNER["all"], in_maps, cfg)
    tlog.append(("run", time.time()))
    if os.environ.get("GAT_TIMING"):
        for (nm, t1), (_, t0) in zip(tlog[1:], tlog[:-1]):
            print(f"  [kernel] {nm}: {t1-t0:.2f}s", file=sys.stderr)
    _LAST_DEV_IN = dev_in

    outT = np.concatenate([res[c]["outT"] for c in range(cfg.NC)], axis=1)
    out = outT.T[lab[: cfg.N]]
    return np.ascontiguousarray(out, np.float32)


def bench(n=5, pipeline=12):
    """Re-execute the compiled program on device-resident inputs.

    Returns (serial_walls, pipelined_per_call): serial walls include the
    axon dispatch round trip; the pipelined estimate enqueues `pipeline`
    executions and blocks once, so per-call ~= device execution time.
    Sets LAST_EXEC_NS to the pipelined estimate."""
    global LAST_EXEC_NS
    import jax
    cfg = FULL
    pack = _RUNNER["all"]
    dev_in = _LAST_DEV_IN
    args = [dev_in[nm] for nm in pack["in_names"]]
    nz = n + 2 * pipeline
    zsets = [[jax.device_put(np.zeros((cfg.NC * s[0], *s[1:]), d))
              for s, d in pack["zero_shapes"]] for _ in range(nz + 1)]
    jax.block_until_ready(zsets)
    o = pack["fn"](*args, *zsets[0])
    jax.block_until_ready(o)
    times = []
    for r in range(1, n + 1):
        t0 = time.perf_counter()
        o = pack["fn"](*args, *zsets[r])
        jax.block_until_ready(o)
        times.append(time.perf_counter() - t0)
    # pipelined: enqueue `pipeline` calls, block once; do twice, take min
    per_call = []
    for rep in range(2):
        zs = zsets[n + 1 + rep * pipeline : n + 1 + (rep + 1) * pipeline]
        t0 = time.perf_counter()
        outs = [pack["fn"](*args, *z) for z in zs]
        jax.block_until_ready(outs)
        dt = time.perf_counter() - t0
        per_call.append((dt - min(times)) / max(1, pipeline - 1))
    est = max(min(per_call), 0.0)
    LAST_EXEC_NS = int(est * 1e9)
    return times, per_call
